# revision 12
# baseline (speedup 1.0000x reference)
"""Trainium2 Bass kernel for nn_PhaseAdaptiveInput (embedding lookup).

out[b] = act(sum_f W[feature_indices[b,f], bucket(b)*256:(bucket(b)+1)*256] + bias_bucket)
with bucket(b) = ply[b] // 7 and act(x) = clip(x,0,1)^2 * 255/256.

Strategy (8 NeuronCores, data parallel over samples, bucket-sharded):
  - Samples grouped by bucket host-side; core k gets bucket k's samples
    (~1024). All cores run ONE static SPMD program; the per-core bucket
    enters via the W input, a numpy VIEW of the (host-converted bf16) flat
    table starting at element 256*k. bf16 halves the gather bytes; rel err
    stays ~5e-3 (budget 2e-2).
  - Gathers are organized as per-(128-sample block, table half) segments,
    statically sized to the max count across the 8 cores (rounded to 128),
    split into near-uniform sub-calls of <= 896 indices (single_packet
    requires <= 63 descriptors/engine). Sub-calls are issued round-robin on
    4 SWDGE queues so descriptor generation runs concurrently on 4 Q7 core
    pairs (single-queue Q7 descgen is ~9ns/static-idx and is the kernel's
    bottleneck; per-call cost is linear in static num_idxs).
  - Rows split by half (row < 32768 vs >= 32768) because gather indices are
    signed int16; the upper half uses a W view offset by 32768 rows.
  - EVERY call is "full": segment tails are padded with index 0 (a valid
    row; its mask entry is -1 so the matmul ignores it). No per-call counts,
    no value_loads, no zero-fill DMAs -- the Pool engine's in-order queue
    then never stalls on scalar loads or prefill DMAs between descgens.
  - A warmup gather (128 x row 0 from a memset idx tile, queue 0) issues
    right after load_library so the ~6us one-time IRAM load of the gather
    ucode overlaps the constant-table DMAs instead of delaying call 0.
  - The index table is split into 4 tiles loaded by 4 separate DMAs so the
    first gather only waits on the first quarter.
  - Per 128-slot chunk a -1/0..127 bf16 owner column is compared against an
    iota row on DVE (is_equal) to build the [slot, sample] mask; chunk
    matmuls (bf16 -> f32 PSUM) accumulate per-sample sums per block.
  - Epilogue per block: +bias on DVE, then clip(x,0,1) = relu(x)-relu(x-1)
    and square*255/256 (= Square(x*sqrt(s))) on the ACT engine.

The program is compiled on first call, specialized to the input's segment
sizes; recompiled only if a later input changes the layout.

Self-contained: hardcodes all shapes for the 8192x32 / 65536x2048 problem.
"""
import sys
import numpy as np

for _p in ("/opt/trn_rl_repo", "/root/.axon_site/_ro/trn_rl_repo"):
    if _p not in sys.path:
        sys.path.append(_p)

# ---------------------------------------------------------------- constants
BATCH = 8192
NFEAT = 32
NROWS = 65536
COUNT = 8
ODIM = 256
BUCKET_SIZE = 7
ACT_SCALE = 255.0 / 256.0
ROW_STRIDE = 2048          # f32 elements per table row
NH = 32768                 # rows per int16-addressable half
SUBCAP = 896               # max idxs per gather sub-call (single_packet: <=63 descs/engine)
NQUEUES = 4
NIDX_TILES = 4             # idx table split into this many separately-loaded tiles
W_LEN = (2 * NH - 1) * ROW_STRIDE + ODIM   # per-core view length
GATHER_BUFS = 22
MASK_BUFS = 16

_compiled = None           # (nc, key)


def _plan_layout(seg_sizes):
    """seg_sizes: list over (block, half) in stream order of static sizes
    (multiples of 128, possibly 0). Returns the static call plan."""
    # sub-call lists per (block, half), near-uniform multiple-of-128 sizes
    per_seg = {}  # (b, h) -> [(size, seg_off), ...]
    for b in range(len(seg_sizes) // 2):
        for h in (0, 1):
            seg = seg_sizes[2 * b + h]
            if seg == 0:
                continue
            n_sub = -(-seg // SUBCAP)
            units = seg // 128
            per, rem = divmod(units, n_sub)
            sizes = [(per + 1) * 128] * rem + [per * 128] * (n_sub - rem)
            off = 0
            lst = []
            for size in sizes:
                lst.append((size, off))
                off += size
            per_seg[(b, h)] = lst

    # Emission order: per PAIR of consecutive used blocks, the full-size
    # (non-tail) calls of all four segments first, then the smaller tail
    # calls together — so each queue group-of-4 carries uniform sizes and no
    # Q7 pair idles waiting for a longer sibling call to retire. An odd
    # block (the short last one) goes FIRST so the kernel does not end on a
    # block whose entire data arrives last; the final pair's tails are
    # re-split to <=384 so the chunk->matmul->epilogue chain gated by the
    # very last calls is short.
    blocks = sorted({b for b, h in per_seg})
    groups = []
    if len(blocks) % 2:
        groups.append(blocks[-1:])
        blocks = blocks[:-1]
    groups.extend(blocks[i:i + 2] for i in range(0, len(blocks), 2))
    ordered = []  # (b, h, size, seg_off)
    for gi, pair in enumerate(groups):
        body, tails, fin = [], [], []
        for b in pair:
            for h in (0, 1):
                lst = per_seg.get((b, h), [])
                if not lst:
                    continue
                body.extend((b, h, s, o) for s, o in lst[:-1])
                size, off = lst[-1]
                if gi == len(groups) - 1 and size >= 512:
                    fine = size - 256
                    tails.append((b, h, fine, off))
                    fin.append((b, h, 256, off + fine))
                else:
                    tails.append((b, h, size, off))
        ordered.extend(body)
        ordered.extend(tails)
        ordered.extend(fin)

    calls = []  # (block, half, size, seg_off, icol_off, chunk_off, queue)
    icol_off = 0
    chunk_off = 0
    for ci, (b, h, size, seg_off) in enumerate(ordered):
        calls.append((b, h, size, seg_off, icol_off, chunk_off,
                      (ci + 1) % NQUEUES))
        icol_off += size // 16
        chunk_off += size // 128
    return calls


def _idx_tile_splits(calls):
    """Partition the call list into NIDX_TILES contiguous groups of roughly
    equal icol footprint. Returns list of (call_lo, call_hi, icol_lo, ncols)."""
    total_icol = sum(c[2] for c in calls) // 16
    splits = []
    target = -(-total_icol // NIDX_TILES)
    lo = 0
    while lo < len(calls):
        icol_lo = calls[lo][4]
        hi = lo
        while hi < len(calls) and calls[hi][4] + calls[hi][2] // 16 - icol_lo <= target:
            hi += 1
        if hi == lo:
            hi = lo + 1
        icol_hi = calls[hi - 1][4] + calls[hi - 1][2] // 16
        splits.append((lo, hi, icol_lo, icol_hi - icol_lo))
        lo = hi
    return splits


def _build_program(seg_sizes):
    import concourse.bacc as bacc
    import concourse.bass as bass
    import concourse.mybir as mybir
    import concourse.tile as tile
    from concourse.library_config import mlp

    F32 = mybir.dt.float32
    BF16 = mybir.dt.bfloat16

    calls = _plan_layout(seg_sizes)
    nblocks = len(seg_sizes) // 2
    splits = _idx_tile_splits(calls)
    total_chunks = sum(c[2] for c in calls) // 128
    # per-block chunk lists (emission order) for PSUM start/stop; chunks
    # alternate between two PSUM accumulators so back-to-back matmuls hit
    # different PSUM banks and pipeline instead of serializing.
    blk_chunks = {}
    for b, h, size, so, io, co, q in calls:
        for j in range(size // 128):
            blk_chunks.setdefault(b, []).append(co + j)
    blk_first = {b: c[0] for b, c in blk_chunks.items()}
    blk_last = {b: c[-1] for b, c in blk_chunks.items()}
    # accumulator id per chunk (index within the block's chunk list % 2;
    # single-chunk blocks stay on accumulator 0)
    acc_of = {}
    acc_first = {}
    acc_last = {}
    for b, chs in blk_chunks.items():
        nacc = 2 if len(chs) >= 2 else 1
        for i, ch in enumerate(chs):
            a = i % nacc
            acc_of[ch] = a
            acc_first.setdefault((b, a), ch)
            acc_last[(b, a)] = ch
        blk_chunks[b] = (chs, nacc)

    nc = bacc.Bacc("TRN2", target_bir_lowering=False, debug=False,
                   num_swdge_queues=NQUEUES)
    w = nc.dram_tensor("w", [W_LEN], BF16, kind="ExternalInput")
    idxs_d = [nc.dram_tensor(f"idxs{t}", [128, max(16, ncols)],
                             mybir.dt.int16, kind="ExternalInput")
              for t, (_, _, _, ncols) in enumerate(splits)]
    owners_d = nc.dram_tensor("owners", [128, max(8, total_chunks)], F32,
                              kind="ExternalInput")
    bias_d = nc.dram_tensor("biasrep", [128, ODIM], F32, kind="ExternalInput")
    iota_d = nc.dram_tensor("iota", [128, 128], BF16, kind="ExternalInput")
    out_d = nc.dram_tensor("out", [nblocks * 128, ODIM], F32,
                           kind="ExternalOutput")
    wt = w[:].tensor

    with tile.TileContext(nc) as tc:
        with tc.tile_pool(name="const", bufs=1) as cpool, \
             tc.tile_pool(name="gather", bufs=GATHER_BUFS) as gpool, \
             tc.tile_pool(name="mask", bufs=MASK_BUFS) as mpool, \
             tc.tile_pool(name="acts", bufs=4) as apool, \
             tc.tile_pool(name="psum", bufs=8, space="PSUM") as pspool:
            nc.gpsimd.load_library(mlp)

            # Warmup gather: loads the gather ucode IRAM (~6us) and queue-0
            # setup concurrently with the constant DMAs below. Reads row 0
            # x128 from a memset idx tile into a scratch tile nothing reads.
            widx_t = cpool.tile([128, 16], mybir.dt.int16, tag="widx")
            nc.vector.memset(widx_t[:], 0)
            warm_dst = cpool.tile([128, 1, ODIM], BF16, tag="wdst")
            w_view0 = bass.AP(tensor=wt, offset=0,
                              ap=[(ROW_STRIDE, NH), (1, ODIM)])
            nc.gpsimd.dma_gather(
                warm_dst[:, :1, :], w_view0, widx_t[:, :8],
                128, 128, ODIM, elem_step=ROW_STRIDE,
                single_packet=True, queue_num=0)

            idx_ts = []
            for t, (_, _, _, ncols) in enumerate(splits):
                it = cpool.tile([128, max(16, ncols)], mybir.dt.int16,
                                tag=f"idx{t}")
                nc.sync.dma_start(it[:, :], idxs_d[t][:, :])
                idx_ts.append(it)
            own_t = cpool.tile([128, max(8, total_chunks)], F32, tag="own")
            bias_t = cpool.tile([128, ODIM], F32, tag="bias")
            iota_t = cpool.tile([128, 128], BF16, tag="iota")
            nc.sync.dma_start(own_t[:, :], owners_d[:, :])
            nc.sync.dma_start(bias_t[:, :], bias_d[:, :])
            nc.sync.dma_start(iota_t[:, :], iota_d[:, :])
            zero_s = cpool.tile([128, 1], F32, tag="zs")
            negone_s = cpool.tile([128, 1], F32, tag="ns")
            nc.vector.memset(zero_s[:], 0.0)
            nc.vector.memset(negone_s[:], -1.0)

            psum_tiles = {}
            for ti, (clo, chi, icol_lo, _) in enumerate(splits):
                idx_t = idx_ts[ti]
                for b, h, size, so, io, co, q in calls[clo:chi]:
                    schunk = size // 128
                    dst = gpool.tile([128, SUBCAP // 128, ODIM], BF16,
                                     tag="dst")
                    w_view = bass.AP(tensor=wt, offset=h * NH * ROW_STRIDE,
                                     ap=[(ROW_STRIDE, NH), (1, ODIM)])
                    rel = io - icol_lo
                    nc.gpsimd.dma_gather(
                        dst[:, :schunk, :], w_view,
                        idx_t[:, rel:rel + size // 16],
                        size, size, ODIM, elem_step=ROW_STRIDE,
                        single_packet=True, queue_num=q)

                    # mask[p, j, s] = (iota[p, s] == own[p, chunk]) built as a
                    # per-chunk tensor_scalar (per-partition scalar operand):
                    # single-stream DVE op, so it does not hold the 2-input
                    # DVE port mode that contends with Q7 SWDGE SBUF access.
                    mask = mpool.tile([128, SUBCAP // 128, 128], BF16,
                                      tag="mask")
                    for j in range(schunk):
                        nc.vector.tensor_scalar(
                            mask[:, j, :], iota_t[:, :],
                            own_t[:, co + j:co + j + 1], None,
                            mybir.AluOpType.is_equal)

                    if blk_first[b] == co:
                        nacc = blk_chunks[b][1]
                        psum_tiles[b] = [
                            pspool.tile([128, ODIM], F32, tag="ps",
                                        name=f"ps{b}a{a}")
                            for a in range(nacc)]
                    for j in range(schunk):
                        ch = co + j
                        a = acc_of[ch]
                        pt = psum_tiles[b][a]
                        nc.tensor.matmul(pt[:],
                                         lhsT=mask[:, j, :],
                                         rhs=dst[:, j, :],
                                         start=(ch == acc_first[(b, a)]),
                                         stop=(ch == acc_last[(b, a)]))
                        if ch == blk_last[b]:
                            # clip(x,0,1) == relu(x) - relu(x-1); then
                            # square-and-scale via Square(d*sqrt(s)) = d^2*s.
                            # relu/square run on the (otherwise idle) ACT
                            # engine.
                            act = apool.tile([128, ODIM], F32, tag="act")
                            r1 = apool.tile([128, ODIM], F32, tag="r1")
                            pts = psum_tiles[b]
                            nc.vector.tensor_add(act[:], pts[0][:],
                                                 bias_t[:])
                            if len(pts) == 2:
                                # only one tensor_tensor input may be PSUM
                                nc.vector.tensor_tensor(
                                    act[:], act[:], pts[1][:],
                                    mybir.AluOpType.add)
                            nc.scalar.activation(
                                r1[:], act[:],
                                mybir.ActivationFunctionType.Relu,
                                bias=negone_s[:, :1])
                            nc.scalar.activation(
                                act[:], act[:],
                                mybir.ActivationFunctionType.Relu,
                                bias=zero_s[:, :1])
                            nc.vector.tensor_sub(act[:], act[:], r1[:])
                            nc.scalar.activation(
                                act[:], act[:],
                                mybir.ActivationFunctionType.Square,
                                bias=zero_s[:, :1],
                                scale=float(ACT_SCALE ** 0.5))
                            nc.sync.dma_start(
                                out_d[b * 128:(b + 1) * 128, :], act[:])
            # blocks with zero static size still need defined output rows
            for b in range(nblocks):
                if b not in blk_first:
                    act = apool.tile([128, ODIM], F32, tag="act")
                    nc.vector.memset(act[:], 0.0)
                    nc.sync.dma_start(out_d[b * 128:(b + 1) * 128, :], act[:])
    nc.compile()
    return nc


def _host_prep(feature_indices, ply):
    """Returns (seg_sizes, per-core data dicts, samp_ids)."""
    fi = np.asarray(feature_indices, dtype=np.int64)
    plyv = np.asarray(ply, dtype=np.int64)
    bucket = np.clip(plyv // BUCKET_SIZE, 0, COUNT - 1)

    samp_ids = []
    core_ents = []  # per core: list over blocks of [(idx_h0, own_h0), (idx_h1, own_h1)]
    nmax = 0
    for k in range(COUNT):
        samp = np.nonzero(bucket == k)[0]
        samp_ids.append(samp)
        nmax = max(nmax, len(samp))
    nblocks = max(1, -(-nmax // 128))

    for k in range(COUNT):
        samp = samp_ids[k]
        n = len(samp)
        rows = fi[samp]  # [n, 32]
        blocks = []
        for b in range(nblocks):
            lo, hi = b * 128, min(n, b * 128 + 128)
            if hi > lo:
                r = rows[lo:hi]
                owner = np.repeat(np.arange(hi - lo), NFEAT)
                rflat = r.reshape(-1)
                half = rflat >= NH
                per_half = []
                for h in (0, 1):
                    sel = np.nonzero(half == bool(h))[0]
                    per_half.append((rflat[sel] - h * NH,
                                     owner[sel].astype(np.float32)))
                blocks.append(per_half)
            else:
                z = (np.zeros(0, np.int64), np.zeros(0, np.float32))
                blocks.append([z, z])
        core_ents.append(blocks)

    # static segment sizes: max over cores, rounded up to 128
    seg_sizes = []
    for b in range(nblocks):
        for h in (0, 1):
            m = max(len(core_ents[k][b][h][0]) for k in range(COUNT))
            seg_sizes.append(-(-m // 128) * 128)

    calls = _plan_layout(seg_sizes)
    splits = _idx_tile_splits(calls)
    total_chunks = sum(c[2] for c in calls) // 128
    import ml_dtypes
    iota = np.broadcast_to(np.arange(128, dtype=np.float32),
                           (128, 128)).astype(ml_dtypes.bfloat16)

    cores = []
    for k in range(COUNT):
        # pad slots: owner -1 (mask column all-zero -> contributes nothing
        # to the matmul); the pad index value is striped across rows (all
        # rows are "safe") -- identical pad indices would hammer one HBM
        # page and serialize the tail DMA drain.
        idx_arrs = [np.zeros((128, max(16, ncols)), np.int16)
                    for (_, _, _, ncols) in splits]
        own_arr = np.full((128, max(8, total_chunks)), -1.0,
                          np.float32)
        for ti, (clo, chi, icol_lo, _) in enumerate(splits):
            for b, h, size, seg_off, io, co, q in calls[clo:chi]:
                ents, owns = core_ents[k][b][h]
                seg = ents[seg_off:seg_off + size]
                osg = owns[seg_off:seg_off + size]
                m = len(seg)
                col = (np.arange(size, dtype=np.int64) * 661) % NH
                ocol = np.full(size, -1.0, np.float32)
                col[:m] = seg
                ocol[:m] = osg
                wrap = col.reshape(size // 16, 16).T.astype(np.int16)
                rel = io - icol_lo
                idx_arrs[ti][:, rel:rel + size // 16] = np.tile(wrap, (8, 1))
                own_arr[:, co:co + size // 128] = \
                    ocol.reshape(size // 128, 128).T
        core = {f"idxs{t}": idx_arrs[t] for t in range(len(splits))}
        core["owners"] = own_arr
        core["iota"] = iota
        cores.append(core)
    return seg_sizes, cores, samp_ids


def _fallback(feature_indices, ply, W, bias):
    fi = np.asarray(feature_indices, dtype=np.int64)
    plyv = np.asarray(ply, dtype=np.int64)
    bucket = np.clip(plyv // BUCKET_SIZE, 0, COUNT - 1)
    Wr = np.asarray(W, dtype=np.float32).reshape(NROWS, COUNT, ODIM)
    br = np.asarray(bias, np.float32).reshape(COUNT, ODIM)
    out = np.empty((len(plyv), ODIM), np.float32)
    for b in range(len(plyv)):
        acc = Wr[fi[b], bucket[b], :].sum(axis=0) + br[bucket[b]]
        out[b] = np.clip(acc, 0.0, 1.0) ** 2 * ACT_SCALE
    return out


def _make_in_maps(cores, W, bias):
    import ml_dtypes
    wflat = W.reshape(-1).astype(ml_dtypes.bfloat16)
    biasr = bias.reshape(COUNT, ODIM)
    in_maps = []
    for k in range(COUNT):
        m = dict(cores[k])
        m["w"] = wflat[k * ODIM: k * ODIM + W_LEN]
        m["biasrep"] = np.broadcast_to(biasr[k], (128, ODIM)).copy()
        in_maps.append(m)
    return in_maps


def _prepare(feature_indices, ply):
    """Host prep + (re)compile. Returns (nc, cores, samp_ids) or None if the
    input shape is outside the static plan (caller should fall back)."""
    global _compiled
    seg_sizes, cores, samp_ids = _host_prep(feature_indices, ply)
    if max(len(s) for s in samp_ids) > 4096:
        return None
    key = tuple(seg_sizes)
    if _compiled is None or _compiled[1] != key:
        _compiled = (_build_program(seg_sizes), key)
    return _compiled[0], cores, samp_ids


def kernel(feature_indices, ply, W, bias):
    from concourse.bass_utils import run_bass_kernel_spmd

    W = np.ascontiguousarray(np.asarray(W, dtype=np.float32))
    bias = np.asarray(bias, dtype=np.float32)
    prep = _prepare(feature_indices, ply)
    if prep is None:
        return _fallback(feature_indices, ply, W, bias)
    nc, cores, samp_ids = prep
    in_maps = _make_in_maps(cores, W, bias)
    res = run_bass_kernel_spmd(nc, in_maps, core_ids=list(range(COUNT)))
    out = np.empty((BATCH, ODIM), np.float32)
    for k in range(COUNT):
        ids = samp_ids[k]
        out[ids] = res.results[k]["out"][: len(ids)]
    return out


# revision 14
# speedup vs baseline: 1.5300x; 1.5300x over previous
"""Trainium2 Bass kernel for nn_PhaseAdaptiveInput (embedding lookup).

out[b] = act(sum_f W[feature_indices[b,f], bucket(b)*256:(bucket(b)+1)*256] + bias_bucket)
with bucket(b) = ply[b] // 7 and act(x) = clip(x,0,1)^2 * 255/256.

Strategy (8 NeuronCores, data parallel over samples, bucket-sharded):
  - Samples grouped by bucket host-side; core k gets bucket k's samples
    (~1024). All cores run ONE static SPMD program; the per-core bucket
    enters via the W input, a numpy VIEW of the (host-converted bf16) flat
    table starting at element 256*k. bf16 halves the gather bytes; rel err
    stays ~5e-3 (budget 2e-2).
  - Gathers are organized as per-(128-sample block, table half) segments,
    statically sized to the max count across the 8 cores (rounded to 128),
    split into near-uniform sub-calls of <= 896 indices (single_packet
    requires <= 63 descriptors/engine). Sub-calls are issued round-robin on
    4 SWDGE queues so descriptor generation runs concurrently on 4 Q7 core
    pairs (single-queue Q7 descgen is ~9ns/static-idx and is the kernel's
    bottleneck; per-call cost is linear in static num_idxs).
  - Rows split by half (row < 32768 vs >= 32768) because gather indices are
    signed int16; the upper half uses a W view offset by 32768 rows.
  - EVERY call is "full": segment tails are padded with index 0 (a valid
    row; its mask entry is -1 so the matmul ignores it). No per-call counts,
    no value_loads, no zero-fill DMAs -- the Pool engine's in-order queue
    then never stalls on scalar loads or prefill DMAs between descgens.
  - A warmup gather (128 x row 0 from a memset idx tile, queue 0) issues
    right after load_library so the ~6us one-time IRAM load of the gather
    ucode overlaps the constant-table DMAs instead of delaying call 0.
  - The index table is split into 4 tiles loaded by 4 separate DMAs so the
    first gather only waits on the first quarter.
  - Per 128-slot chunk a -1/0..127 bf16 owner column is compared against an
    iota row on DVE (is_equal) to build the [slot, sample] mask; chunk
    matmuls (bf16 -> f32 PSUM) accumulate per-sample sums per block.
  - Epilogue per block: +bias on DVE, then clip(x,0,1) = relu(x)-relu(x-1)
    and square*255/256 (= Square(x*sqrt(s))) on the ACT engine.

The program is compiled on first call, specialized to the input's segment
sizes; recompiled only if a later input changes the layout.

Self-contained: hardcodes all shapes for the 8192x32 / 65536x2048 problem.
"""
import sys
import numpy as np

for _p in ("/opt/trn_rl_repo", "/root/.axon_site/_ro/trn_rl_repo"):
    if _p not in sys.path:
        sys.path.append(_p)

# ---------------------------------------------------------------- constants
BATCH = 8192
NFEAT = 32
NROWS = 65536
COUNT = 8
ODIM = 256
BUCKET_SIZE = 7
ACT_SCALE = 255.0 / 256.0
ROW_STRIDE = 2048          # f32 elements per table row
NH = 32768                 # rows per int16-addressable half
SUBCAP = 896               # max idxs per gather sub-call (single_packet: <=63 descs/engine)
NQUEUES = 4
NIDX_TILES = 4             # idx table split into this many separately-loaded tiles
W_LEN = (2 * NH - 1) * ROW_STRIDE + ODIM   # per-core view length
GATHER_BUFS = 22
MASK_BUFS = 16

_compiled = None           # (nc, key)


def _plan_layout(seg_sizes):
    """seg_sizes: list over (block, half) in stream order of static sizes
    (multiples of 128, possibly 0). Returns the static call plan."""
    # sub-call lists per (block, half), near-uniform multiple-of-128 sizes
    per_seg = {}  # (b, h) -> [(size, seg_off), ...]
    for b in range(len(seg_sizes) // 2):
        for h in (0, 1):
            seg = seg_sizes[2 * b + h]
            if seg == 0:
                continue
            n_sub = -(-seg // SUBCAP)
            units = seg // 128
            per, rem = divmod(units, n_sub)
            sizes = [(per + 1) * 128] * rem + [per * 128] * (n_sub - rem)
            off = 0
            lst = []
            for size in sizes:
                lst.append((size, off))
                off += size
            per_seg[(b, h)] = lst

    # Emission order: per PAIR of consecutive used blocks, the full-size
    # (non-tail) calls of all four segments first, then the smaller tail
    # calls together — so each queue group-of-4 carries uniform sizes and no
    # Q7 pair idles waiting for a longer sibling call to retire. An odd
    # block (the short last one) goes FIRST so the kernel does not end on a
    # block whose entire data arrives last; the final pair's tails are
    # re-split to <=384 so the chunk->matmul->epilogue chain gated by the
    # very last calls is short.
    blocks = sorted({b for b, h in per_seg})
    groups = []
    if len(blocks) % 2:
        groups.append(blocks[-1:])
        blocks = blocks[:-1]
    groups.extend(blocks[i:i + 2] for i in range(0, len(blocks), 2))
    ordered = []  # (b, h, size, seg_off)
    for gi, pair in enumerate(groups):
        body, tails, fin = [], [], []
        for b in pair:
            for h in (0, 1):
                lst = per_seg.get((b, h), [])
                if not lst:
                    continue
                body.extend((b, h, s, o) for s, o in lst[:-1])
                size, off = lst[-1]
                if gi == len(groups) - 1 and size >= 512:
                    fine = size - 256
                    tails.append((b, h, fine, off))
                    fin.append((b, h, 256, off + fine))
                else:
                    tails.append((b, h, size, off))
        ordered.extend(body)
        ordered.extend(tails)
        ordered.extend(fin)

    calls = []  # (block, half, size, seg_off, icol_off, chunk_off, queue)
    icol_off = 0
    chunk_off = 0
    for ci, (b, h, size, seg_off) in enumerate(ordered):
        calls.append((b, h, size, seg_off, icol_off, chunk_off,
                      (ci + 1) % NQUEUES))
        icol_off += size // 16
        chunk_off += size // 128
    return calls


def _idx_tile_splits(calls):
    """Partition the call list into NIDX_TILES contiguous groups of roughly
    equal icol footprint. Returns list of (call_lo, call_hi, icol_lo, ncols)."""
    total_icol = sum(c[2] for c in calls) // 16
    splits = []
    target = -(-total_icol // NIDX_TILES)
    lo = 0
    while lo < len(calls):
        icol_lo = calls[lo][4]
        hi = lo
        while hi < len(calls) and calls[hi][4] + calls[hi][2] // 16 - icol_lo <= target:
            hi += 1
        if hi == lo:
            hi = lo + 1
        icol_hi = calls[hi - 1][4] + calls[hi - 1][2] // 16
        splits.append((lo, hi, icol_lo, icol_hi - icol_lo))
        lo = hi
    return splits


def _build_program(seg_sizes):
    import concourse.bacc as bacc
    import concourse.bass as bass
    import concourse.mybir as mybir
    import concourse.tile as tile
    from concourse.library_config import mlp

    F32 = mybir.dt.float32
    BF16 = mybir.dt.bfloat16

    calls = _plan_layout(seg_sizes)
    nblocks = len(seg_sizes) // 2
    splits = _idx_tile_splits(calls)
    total_chunks = sum(c[2] for c in calls) // 128
    # per-block chunk lists (emission order) for PSUM start/stop; chunks
    # alternate between two PSUM accumulators so back-to-back matmuls hit
    # different PSUM banks and pipeline instead of serializing.
    blk_chunks = {}
    for b, h, size, so, io, co, q in calls:
        for j in range(size // 128):
            blk_chunks.setdefault(b, []).append(co + j)
    blk_first = {b: c[0] for b, c in blk_chunks.items()}
    blk_last = {b: c[-1] for b, c in blk_chunks.items()}
    # accumulator id per chunk (index within the block's chunk list % 2;
    # single-chunk blocks stay on accumulator 0)
    acc_of = {}
    acc_first = {}
    acc_last = {}
    for b, chs in blk_chunks.items():
        nacc = 2 if len(chs) >= 2 else 1
        for i, ch in enumerate(chs):
            a = i % nacc
            acc_of[ch] = a
            acc_first.setdefault((b, a), ch)
            acc_last[(b, a)] = ch
        blk_chunks[b] = (chs, nacc)

    nc = bacc.Bacc("TRN2", target_bir_lowering=False, debug=False,
                   num_swdge_queues=NQUEUES)
    w = nc.dram_tensor("w", [W_LEN], BF16, kind="ExternalInput")
    idxs_d = [nc.dram_tensor(f"idxs{t}", [128, max(16, ncols)],
                             mybir.dt.int16, kind="ExternalInput")
              for t, (_, _, _, ncols) in enumerate(splits)]
    owners_d = nc.dram_tensor("owners", [128, max(8, total_chunks)], BF16,
                              kind="ExternalInput")
    bias_d = nc.dram_tensor("biasrep", [128, ODIM], F32, kind="ExternalInput")
    iota_d = nc.dram_tensor("iota", [128, 128], BF16, kind="ExternalInput")
    out_d = nc.dram_tensor("out", [nblocks * 128, ODIM], F32,
                           kind="ExternalOutput")
    wt = w[:].tensor

    with tile.TileContext(nc) as tc:
        with tc.tile_pool(name="const", bufs=1) as cpool, \
             tc.tile_pool(name="gather", bufs=GATHER_BUFS) as gpool, \
             tc.tile_pool(name="mask", bufs=MASK_BUFS) as mpool, \
             tc.tile_pool(name="acts", bufs=4) as apool, \
             tc.tile_pool(name="psum", bufs=8, space="PSUM") as pspool:
            nc.gpsimd.load_library(mlp)

            # Warmup gather: loads the gather ucode IRAM (~6us) and queue-0
            # setup concurrently with the constant DMAs below. Reads row 0
            # x128 from a memset idx tile into a scratch tile nothing reads.
            widx_t = cpool.tile([128, 16], mybir.dt.int16, tag="widx")
            nc.vector.memset(widx_t[:], 0)
            warm_dst = cpool.tile([128, 1, ODIM], BF16, tag="wdst")
            w_view0 = bass.AP(tensor=wt, offset=0,
                              ap=[(ROW_STRIDE, NH), (1, ODIM)])
            nc.gpsimd.dma_gather(
                warm_dst[:, :1, :], w_view0, widx_t[:, :8],
                128, 128, ODIM, elem_step=ROW_STRIDE,
                single_packet=True, queue_num=0)

            idx_ts = []
            for t, (_, _, _, ncols) in enumerate(splits):
                it = cpool.tile([128, max(16, ncols)], mybir.dt.int16,
                                tag=f"idx{t}")
                nc.sync.dma_start(it[:, :], idxs_d[t][:, :])
                idx_ts.append(it)
            own_t = cpool.tile([128, max(8, total_chunks)], BF16, tag="own")
            bias_t = cpool.tile([128, ODIM], F32, tag="bias")
            iota_t = cpool.tile([128, 128], BF16, tag="iota")
            nc.sync.dma_start(own_t[:, :], owners_d[:, :])
            nc.sync.dma_start(bias_t[:, :], bias_d[:, :])
            nc.sync.dma_start(iota_t[:, :], iota_d[:, :])
            zero_s = cpool.tile([128, 1], F32, tag="zs")
            negone_s = cpool.tile([128, 1], F32, tag="ns")
            nc.vector.memset(zero_s[:], 0.0)
            nc.vector.memset(negone_s[:], -1.0)

            psum_tiles = {}
            for ti, (clo, chi, icol_lo, _) in enumerate(splits):
                idx_t = idx_ts[ti]
                for b, h, size, so, io, co, q in calls[clo:chi]:
                    schunk = size // 128
                    dst = gpool.tile([128, SUBCAP // 128, ODIM], BF16,
                                     tag="dst")
                    w_view = bass.AP(tensor=wt, offset=h * NH * ROW_STRIDE,
                                     ap=[(ROW_STRIDE, NH), (1, ODIM)])
                    rel = io - icol_lo
                    nc.gpsimd.dma_gather(
                        dst[:, :schunk, :], w_view,
                        idx_t[:, rel:rel + size // 16],
                        size, size, ODIM, elem_step=ROW_STRIDE,
                        single_packet=True, queue_num=q)

                    mask = mpool.tile([128, SUBCAP // 128, 128], BF16,
                                      tag="mask")
                    own_bc = own_t[:, co:co + schunk] \
                        .unsqueeze(2).to_broadcast([128, schunk, 128])
                    iota_bc = iota_t[:, :].unsqueeze(1) \
                        .to_broadcast([128, schunk, 128])
                    nc.vector.tensor_tensor(mask[:, :schunk, :], own_bc,
                                            iota_bc,
                                            mybir.AluOpType.is_equal)

                    if blk_first[b] == co:
                        nacc = blk_chunks[b][1]
                        psum_tiles[b] = [
                            pspool.tile([128, ODIM], F32, tag="ps",
                                        name=f"ps{b}a{a}")
                            for a in range(nacc)]
                    for j in range(schunk):
                        ch = co + j
                        a = acc_of[ch]
                        pt = psum_tiles[b][a]
                        nc.tensor.matmul(pt[:],
                                         lhsT=mask[:, j, :],
                                         rhs=dst[:, j, :],
                                         start=(ch == acc_first[(b, a)]),
                                         stop=(ch == acc_last[(b, a)]))
                        if ch == blk_last[b]:
                            # clip(x,0,1) == relu(x) - relu(x-1); then
                            # square-and-scale via Square(d*sqrt(s)) = d^2*s.
                            # relu/square run on the (otherwise idle) ACT
                            # engine.
                            act = apool.tile([128, ODIM], F32, tag="act")
                            r1 = apool.tile([128, ODIM], F32, tag="r1")
                            pts = psum_tiles[b]
                            nc.vector.tensor_add(act[:], pts[0][:],
                                                 bias_t[:])
                            if len(pts) == 2:
                                # only one tensor_tensor input may be PSUM
                                nc.vector.tensor_tensor(
                                    act[:], act[:], pts[1][:],
                                    mybir.AluOpType.add)
                            nc.scalar.activation(
                                r1[:], act[:],
                                mybir.ActivationFunctionType.Relu,
                                bias=negone_s[:, :1])
                            nc.scalar.activation(
                                act[:], act[:],
                                mybir.ActivationFunctionType.Relu,
                                bias=zero_s[:, :1])
                            nc.vector.tensor_sub(act[:], act[:], r1[:])
                            nc.scalar.activation(
                                act[:], act[:],
                                mybir.ActivationFunctionType.Square,
                                bias=zero_s[:, :1],
                                scale=float(ACT_SCALE ** 0.5))
                            nc.sync.dma_start(
                                out_d[b * 128:(b + 1) * 128, :], act[:])
            # blocks with zero static size still need defined output rows
            for b in range(nblocks):
                if b not in blk_first:
                    act = apool.tile([128, ODIM], F32, tag="act")
                    nc.vector.memset(act[:], 0.0)
                    nc.sync.dma_start(out_d[b * 128:(b + 1) * 128, :], act[:])
    nc.compile()
    return nc


def _host_prep(feature_indices, ply):
    """Returns (seg_sizes, per-core data dicts, samp_ids)."""
    fi = np.asarray(feature_indices, dtype=np.int64)
    plyv = np.asarray(ply, dtype=np.int64)
    bucket = np.clip(plyv // BUCKET_SIZE, 0, COUNT - 1)

    samp_ids = []
    core_ents = []  # per core: list over blocks of [(idx_h0, own_h0), (idx_h1, own_h1)]
    nmax = 0
    for k in range(COUNT):
        samp = np.nonzero(bucket == k)[0]
        samp_ids.append(samp)
        nmax = max(nmax, len(samp))
    nblocks = max(1, -(-nmax // 128))

    for k in range(COUNT):
        samp = samp_ids[k]
        n = len(samp)
        rows = fi[samp]  # [n, 32]
        blocks = []
        for b in range(nblocks):
            lo, hi = b * 128, min(n, b * 128 + 128)
            if hi > lo:
                r = rows[lo:hi]
                owner = np.repeat(np.arange(hi - lo), NFEAT)
                rflat = r.reshape(-1)
                half = rflat >= NH
                per_half = []
                for h in (0, 1):
                    sel = np.nonzero(half == bool(h))[0]
                    per_half.append((rflat[sel] - h * NH,
                                     owner[sel].astype(np.float32)))
                blocks.append(per_half)
            else:
                z = (np.zeros(0, np.int64), np.zeros(0, np.float32))
                blocks.append([z, z])
        core_ents.append(blocks)

    # static segment sizes: max over cores, rounded up to 128
    seg_sizes = []
    for b in range(nblocks):
        for h in (0, 1):
            m = max(len(core_ents[k][b][h][0]) for k in range(COUNT))
            seg_sizes.append(-(-m // 128) * 128)

    calls = _plan_layout(seg_sizes)
    splits = _idx_tile_splits(calls)
    total_chunks = sum(c[2] for c in calls) // 128
    import ml_dtypes
    iota = np.broadcast_to(np.arange(128, dtype=np.float32),
                           (128, 128)).astype(ml_dtypes.bfloat16)

    cores = []
    for k in range(COUNT):
        # pad slots: owner -1 (mask column all-zero -> contributes nothing
        # to the matmul); the pad index value is striped across rows (all
        # rows are "safe") -- identical pad indices would hammer one HBM
        # page and serialize the tail DMA drain.
        idx_arrs = [np.zeros((128, max(16, ncols)), np.int16)
                    for (_, _, _, ncols) in splits]
        own_arr = np.full((128, max(8, total_chunks)), -1.0,
                          ml_dtypes.bfloat16)
        for ti, (clo, chi, icol_lo, _) in enumerate(splits):
            for b, h, size, seg_off, io, co, q in calls[clo:chi]:
                ents, owns = core_ents[k][b][h]
                seg = ents[seg_off:seg_off + size]
                osg = owns[seg_off:seg_off + size]
                m = len(seg)
                col = (np.arange(size, dtype=np.int64) * 661) % NH
                ocol = np.full(size, -1.0, np.float32)
                col[:m] = seg
                ocol[:m] = osg
                wrap = col.reshape(size // 16, 16).T.astype(np.int16)
                rel = io - icol_lo
                idx_arrs[ti][:, rel:rel + size // 16] = np.tile(wrap, (8, 1))
                own_arr[:, co:co + size // 128] = \
                    ocol.reshape(size // 128, 128).T.astype(ml_dtypes.bfloat16)
        core = {f"idxs{t}": idx_arrs[t] for t in range(len(splits))}
        core["owners"] = own_arr
        core["iota"] = iota
        cores.append(core)
    return seg_sizes, cores, samp_ids


def _fallback(feature_indices, ply, W, bias):
    fi = np.asarray(feature_indices, dtype=np.int64)
    plyv = np.asarray(ply, dtype=np.int64)
    bucket = np.clip(plyv // BUCKET_SIZE, 0, COUNT - 1)
    Wr = np.asarray(W, dtype=np.float32).reshape(NROWS, COUNT, ODIM)
    br = np.asarray(bias, np.float32).reshape(COUNT, ODIM)
    out = np.empty((len(plyv), ODIM), np.float32)
    for b in range(len(plyv)):
        acc = Wr[fi[b], bucket[b], :].sum(axis=0) + br[bucket[b]]
        out[b] = np.clip(acc, 0.0, 1.0) ** 2 * ACT_SCALE
    return out


def _make_in_maps(cores, W, bias):
    import ml_dtypes
    wflat = W.reshape(-1).astype(ml_dtypes.bfloat16)
    biasr = bias.reshape(COUNT, ODIM)
    in_maps = []
    for k in range(COUNT):
        m = dict(cores[k])
        m["w"] = wflat[k * ODIM: k * ODIM + W_LEN]
        m["biasrep"] = np.broadcast_to(biasr[k], (128, ODIM)).copy()
        in_maps.append(m)
    return in_maps


def _prepare(feature_indices, ply):
    """Host prep + (re)compile. Returns (nc, cores, samp_ids) or None if the
    input shape is outside the static plan (caller should fall back)."""
    global _compiled
    seg_sizes, cores, samp_ids = _host_prep(feature_indices, ply)
    if max(len(s) for s in samp_ids) > 4096:
        return None
    key = tuple(seg_sizes)
    if _compiled is None or _compiled[1] != key:
        _compiled = (_build_program(seg_sizes), key)
    return _compiled[0], cores, samp_ids


def kernel(feature_indices, ply, W, bias):
    from concourse.bass_utils import run_bass_kernel_spmd

    W = np.ascontiguousarray(np.asarray(W, dtype=np.float32))
    bias = np.asarray(bias, dtype=np.float32)
    prep = _prepare(feature_indices, ply)
    if prep is None:
        return _fallback(feature_indices, ply, W, bias)
    nc, cores, samp_ids = prep
    in_maps = _make_in_maps(cores, W, bias)
    res = run_bass_kernel_spmd(nc, in_maps, core_ids=list(range(COUNT)))
    out = np.empty((BATCH, ODIM), np.float32)
    for k in range(COUNT):
        ids = samp_ids[k]
        out[ids] = res.results[k]["out"][: len(ids)]
    return out
